# revision 1
# baseline (speedup 1.0000x reference)
"""Trainium2 Bass kernel for nn_AttentionAggregator (GNN message passing).

Math (per batch row b, with N=64 neighbors, F=128 in-features, H=8 heads, D=64):
    lin  = x @ W_lin                                      [B, N, 512]
    att  = lin[:,0,:] @ W_att[:512] + lin @ W_att[512:]   [B, N, 8]
    att  = LeakyReLU_0.2(att); masked softmax over N per (b, h)
    out  = relu(lin) * aw                                 [B, N, 512]

Design (final): the dominant cost is the elementwise relu(lin)*aw over the full
output. HW facts (measured): DVE tensor_tensor runs 2x with all-16-bit packed
operands (one may be PSUM); scalar_tensor_tensor never 2x-es; ACT is flat-rate
~1.1us/[128,1024] for any activation incl. Prelu; matmul accumulate groups
take extra terms (mask rank-1, src broadcast-rhs) for free on PE; transpose-
datapath matmuls (is_transpose) write f16 to PSUM when weights select a single
row (routing), which expands aw[8,r] -> awrep[r, 512] in ~150ns.

Per 256-row tile the final multiply runs in one of two balanced pipelines:
  A (DVE-only):  stt out = max(lin_f32psum,0) * aw_sb[128,16] bcast  (~1.19us DVE)
  B (ACT+DVE):   ACT relu lin_f32psum -> rlin f16 sbuf  (~1.1us ACT)
                 DVE tt  rlin * awrep_f16psum            (~0.60us DVE)
The attention chain is mega-batched (16 tiles packed 4-per-32-partition-group):
attA accumulates IN PSUM [128,4,256] via col-offset matmuls incl. the mask
(rank-1 {0,-1e30}) and the src term (wc1.T @ x_src with stride-0 bcast rhs),
then ACT Prelu (leaky, frees PSUM), ACT exp(bias -4), DVE reduce/recip/awmult.
The src term's slot-0 feature columns are prepacked on host (xsrc input) so the
PE consumes them via a stride-0 broadcast rhs with zero engine ops. Next-mega
front matmuls, chain ops and DMA loads are issued inside this mega's backs
stream at dependency-ready points to avoid engine head-of-line blocking.

Sharding: pure data-parallel over batch: 512 batch rows per core
(128 tiles of 256 rows), weights replicated. fp16 I/O; host upcasts.
"""

import os
from contextlib import ExitStack

import ml_dtypes
import numpy as np

import concourse.bacc as bacc
import concourse.bass as bass
import concourse.tile as tile
from concourse import mybir
from concourse.bass_utils import run_bass_kernel_spmd

B, N, F = 4096, 64, 128
H, D = 8, 64
HD = H * D  # 512
NCORES = 8
BSHARD = B // NCORES  # 512
ROWS = BSHARD * N  # 32768
DT_ROWS = 256  # rows per tile (4 batch elements)
DTILES = ROWS // DT_ROWS  # 128
MEGA = 16  # tiles per mega (4 partition groups x 4 slots)
A_SLOTS = (2, 5, 8, 11, 14)  # tiles per mega on pipeline A (DVE stt); rest on B

f32 = mybir.dt.float32
bf16 = mybir.dt.bfloat16
f16 = mybir.dt.float16

LAST_RESULT = None  # test harness reads exec_time_ns / trace from here


def build_nc(dtiles: int = DTILES) -> bass.Bass:
    nc = bacc.Bacc("TRN2", target_bir_lowering=False, debug=False)
    rows = dtiles * DT_ROWS
    assert dtiles % MEGA == 0

    xt = nc.declare_dram_parameter("xt", [dtiles, F, DT_ROWS], f16, isOutput=False)
    xsrc_d = nc.declare_dram_parameter(
        "xsrc", [dtiles // MEGA, F, 4 * MEGA], f16, isOutput=False
    )
    wlin_d = nc.declare_dram_parameter("wlin", [F, HD], f16, isOutput=False)
    watt_d = nc.declare_dram_parameter("watt", [F, 16], f16, isOutput=False)
    ident_d = nc.declare_dram_parameter("ident8", [128, 8], f16, isOutput=False)
    e8rep_d = nc.declare_dram_parameter("e8rep", [128, HD], f16, isOutput=False)
    maskrow_d = nc.declare_dram_parameter("maskrow", [1, rows], bf16, isOutput=False)
    out = nc.declare_dram_parameter("out", [rows, HD], f16, isOutput=True)

    mult = mybir.AluOpType.mult
    mmax = mybir.AluOpType.max

    with tile.TileContext(nc) as tc, ExitStack() as ctx:
        consts = ctx.enter_context(tc.tile_pool(name="consts", bufs=1))
        xin = ctx.enter_context(tc.tile_pool(name="xin", bufs=20))
        xsp = ctx.enter_context(tc.tile_pool(name="xsp", bufs=3))
        rlinp = ctx.enter_context(tc.tile_pool(name="rlinp", bufs=6))
        outp = ctx.enter_context(tc.tile_pool(name="outp", bufs=6))
        small = ctx.enter_context(tc.tile_pool(name="small", bufs=8))
        plin = ctx.enter_context(tc.tile_pool(name="plin", bufs=2, space="PSUM"))
        pawrep = ctx.enter_context(tc.tile_pool(name="pawrep", bufs=2, space="PSUM"))
        pattA = ctx.enter_context(tc.tile_pool(name="pattA", bufs=1, space="PSUM"))

        wlin_sb = consts.tile([F, HD], f16)
        nc.sync.dma_start(out=wlin_sb, in_=wlin_d[:])
        watt_sb = consts.tile([F, 16], f16)
        nc.sync.dma_start(out=watt_sb, in_=watt_d[:])
        # identity blocks + head-expander blocks replicated at partition bases
        # 0/32/64/96 so PE ops share the weight operand's start partition
        ident_sb = consts.tile([128, 8], f16)
        nc.sync.dma_start(out=ident_sb, in_=ident_d[:])
        e8rep_sb = consts.tile([128, HD], f16)
        nc.sync.dma_start(out=e8rep_sb, in_=e8rep_d[:])
        maskrow_sb = consts.tile([1, rows], bf16)
        nc.sync.dma_start(out=maskrow_sb, in_=maskrow_d[:])
        mones_sb = consts.tile([1, 8], bf16)
        nc.vector.memset(mones_sb, 1.0)
        alpha_sb = consts.tile([128, 1], f32)
        nc.vector.memset(alpha_sb, 0.2)
        ebias_sb = consts.tile([128, 1], f32)
        nc.vector.memset(ebias_sb, -4.0)

        # single attA accumulation mega [128, 4, 256] f32 (2 PSUM banks):
        # pair (g, islot..islot+1) writes via tile_position col offset 32g
        attA_ps = pattA.tile([128, 4, DT_ROWS], f32)

        # ping-pong mega slabs (SBUF)
        def mk_slabs(k):
            lk = consts.tile([128, 4 * DT_ROWS], f16, tag=f"slab_lk{k}")
            ew = consts.tile([128, 4 * DT_ROWS], f16, tag=f"slab_ew{k}")
            dn = consts.tile([128, 4, 4, 1], f32, tag=f"slab_dn{k}")
            rd = consts.tile([128, 4, 4, 1], f32, tag=f"slab_rd{k}")
            awt = consts.tile([128, 4, DT_ROWS], f16, tag=f"slab_awt{k}")
            nc.vector.memset(ew, 0.0)
            return lk, ew, dn, rd, awt

        slabs = [mk_slabs(0), mk_slabs(1)]
        nmega = dtiles // MEGA

        def loads(m):
            xs_sb = xsp.tile([F, 4 * MEGA], f16, tag="xs")
            nc.sync.dma_start(out=xs_sb, in_=xsrc_d[m])
            x_tiles = []
            pairs = []
            for j in range(MEGA // 2):
                t = m * MEGA + 2 * j
                x2_sb = xin.tile([F, 2, DT_ROWS], f16, tag="x2")
                nc.sync.dma_start(
                    out=x2_sb,
                    in_=xt[t : t + 2].rearrange("two f r -> f two r"),
                )
                x_tiles.append(x2_sb[:, 0, :])
                x_tiles.append(x2_sb[:, 1, :])
                pairs.append(x2_sb)
            return {"xs": xs_sb, "x_tiles": x_tiles, "pairs": pairs}

        def triplet(m, j, ld):
            i = 2 * j
            t = m * MEGA + i
            g, islot = i // 4, i % 4
            x_pair = ld["pairs"][j].rearrange("f two r -> f (two r)")
            v = attA_ps[32 * g : 32 * g + 8, islot : islot + 2, :].rearrange(
                "h two r -> h (two r)"
            )
            tp = (0, 32 * g)
            nc.tensor.matmul(
                v, watt_sb[:, 0:8], x_pair, start=True, stop=False,
                tile_position=tp, skip_group_check=True,
            )
            nc.tensor.matmul(
                v, mones_sb, maskrow_sb[:, t * DT_ROWS : (t + 2) * DT_ROWS],
                start=False, stop=False, tile_position=tp,
                skip_group_check=True,
            )
            nc.tensor.matmul(
                v.rearrange("h (b n) -> h b n", n=N),
                watt_sb[:, 8:16],
                ld["xs"][:, j * 8 : j * 8 + 8].to_broadcast([F, 8, N]),
                start=False, stop=True, tile_position=tp,
                skip_group_check=True,
            )

        def chain_act(m):
            lk_m, ew_m, den_m, rden_m, awT_m = slabs[m % 2]
            # leaky relu on ACT (frees attA PSUM for the next mega's fronts)
            nc.scalar.activation(
                out=lk_m,
                in_=attA_ps.rearrange("p q r -> p (q r)"),
                func=mybir.ActivationFunctionType.Prelu,
                alpha=alpha_sb,
            )
            nc.scalar.activation(
                out=ew_m, in_=lk_m, func=mybir.ActivationFunctionType.Exp,
                bias=ebias_sb,
            )

        def chain_dve(m):
            lk_m, ew_m, den_m, rden_m, awT_m = slabs[m % 2]
            nc.vector.tensor_reduce(
                out=den_m,
                in_=ew_m.rearrange("p (q b n) -> p q b n", q=4, b=4),
                axis=mybir.AxisListType.X,
                op=mybir.AluOpType.add,
            )
            nc.vector.reciprocal(rden_m, den_m)
            nc.vector.tensor_tensor(
                out=awT_m.rearrange("p q (b n) -> p q b n", n=N),
                in0=ew_m.rearrange("p (q b n) -> p q b n", q=4, b=4),
                in1=rden_m.to_broadcast([128, 4, 4, N]),
                op=mult,
            )

        def aw_prep(m):
            # A-tile aw columns, built once per mega right after awmult so the
            # backs-phase STT only waits on its own lin matmul
            lk_m, ew_m, den_m, rden_m, awT_m = slabs[m % 2]
            pack_ps = pawrep.tile([128, 2, HD], f16, tag="awrep")
            awsb = {}
            for a, i in enumerate(A_SLOTS):
                g, islot = i // 4, i % 4
                for half in range(2):
                    nc.tensor.transpose(
                        pack_ps[:, 0, a * 16 + half * 8 : a * 16 + half * 8 + 8],
                        awT_m[32 * g : 32 * g + 8, islot,
                              half * 128 : half * 128 + 128],
                        ident_sb[32 * g : 32 * g + 8, :],
                        tile_position=(32 * g, 0),
                    )
            for a, i in enumerate(A_SLOTS):
                aw_sb = small.tile([128, 16], f16, tag="aw_sb")
                nc.scalar.copy(out=aw_sb, in_=pack_ps[:, 0, a * 16 : a * 16 + 16])
                awsb[i] = aw_sb
            return awsb

        def backs_tile(m, i, ld, state):
            lk_m, ew_m, den_m, rden_m, awT_m = slabs[m % 2]
            x_tiles = ld["x_tiles"]
            t = m * MEGA + i
            g, islot = i // 4, i % 4
            if i % 2 == 0:
                o2_new = outp.tile([128, 2, 2, HD], f16, tag="o2")
                state["o2"] = o2_new
            o2_sb = state["o2"]
            lin_ps = plin.tile([128, 2, HD], f32, tag="lin")
            nc.tensor.matmul(
                lin_ps[:, 0, :], x_tiles[i][:, 0:128], wlin_sb,
                start=True, stop=True,
            )
            nc.tensor.matmul(
                lin_ps[:, 1, :], x_tiles[i][:, 128:256], wlin_sb,
                start=True, stop=True,
            )
            if i in A_SLOTS:
                aw_ps = pawrep.tile([128, 2, HD], f16, tag="awrep")
                for half in range(2):
                    nc.tensor.transpose(
                        aw_ps[:, 0, half * 8 : half * 8 + 8],
                        awT_m[32 * g : 32 * g + 8, islot,
                              half * 128 : half * 128 + 128],
                        ident_sb[32 * g : 32 * g + 8, :],
                        tile_position=(32 * g, 0),
                    )
                aw_sb = small.tile([128, 16], f16, tag="aw_sb")
                nc.scalar.copy(out=aw_sb, in_=aw_ps[:, 0, 0:16])
                nc.vector.scalar_tensor_tensor(
                    out=o2_sb[:, i % 2].rearrange(
                        "p two (h d) -> p (two h) d", h=H
                    ),
                    in0=lin_ps.rearrange("p two (h d) -> p (two h) d", h=H),
                    scalar=0.0,
                    in1=aw_sb.to_broadcast([128, 2 * H, D]),
                    op0=mmax,
                    op1=mult,
                )
            else:
                # pipeline B: PE head-expander + ACT relu + DVE 2x tt
                aw_ps = pawrep.tile([128, 2, HD], f16, tag="awrep")
                for half in range(2):
                    nc.tensor.matmul(
                        aw_ps[:, half, :],
                        awT_m[32 * g : 32 * g + 8, islot,
                              half * 128 : half * 128 + 128],
                        e8rep_sb[32 * g : 32 * g + 8, :],
                        start=True, stop=True, is_transpose=True,
                        tile_position=(32 * g, 0),
                    )
                rlin_sb = rlinp.tile([128, 2, HD], f16, tag="rlin")
                nc.scalar.activation(
                    out=rlin_sb.rearrange("p two r -> p (two r)"),
                    in_=lin_ps.rearrange("p two r -> p (two r)"),
                    func=mybir.ActivationFunctionType.Relu,
                )
                nc.vector.tensor_tensor(
                    out=o2_sb[:, i % 2], in0=rlin_sb, in1=aw_ps, op=mult,
                )
            if i % 2 == 1:
                out_view = out[
                    (t - 1) * DT_ROWS : (t + 1) * DT_ROWS, :
                ].rearrange("(four p) hd -> p four hd", four=4)
                nc.sync.dma_start(
                    out=out_view,
                    in_=o2_sb.rearrange("p a b hd -> p (a b) hd"),
                )

        # software pipeline: the next mega's att-front matmul triplets, chain
        # ops and A-tile aw prep are threaded INTO this mega's backs stream at
        # points where their dependencies are already satisfied, so no engine
        # queue head-of-line-blocks on a cross-engine wait.
        ld = {0: loads(0)}
        for j in range(MEGA // 2):
            triplet(0, j, ld[0])
        chain_act(0)
        chain_dve(0)
        for m in range(nmega):
            state = {}
            if m + 1 < nmega:
                ld[m + 1] = loads(m + 1)
                for j in range(MEGA // 2):
                    triplet(m + 1, j, ld[m + 1])
            for i in range(MEGA):
                backs_tile(m, i, ld[m], state)
                if m + 1 < nmega:
                    if i == 2:
                        chain_act(m + 1)
                    elif i == 10:
                        chain_dve(m + 1)
            ld.pop(m, None)

    nc.compile()
    return nc


def _host_weights(W_lin, W_att):
    W_lin64 = W_lin.astype(np.float64)
    wc2 = (W_lin64 @ W_att[HD:].astype(np.float64)).astype(np.float32)
    wc1 = (W_lin64 @ W_att[:HD].astype(np.float64)).astype(np.float32)
    watt16 = np.ascontiguousarray(
        np.concatenate([wc2, wc1], axis=1).astype(np.float16)
    )
    ident8 = np.zeros((128, 8), dtype=np.float16)
    e8rep = np.zeros((128, HD), dtype=np.float16)
    for gg in range(4):
        ident8[32 * gg : 32 * gg + 8, :] = np.eye(8, dtype=np.float16)
        for h in range(H):
            e8rep[32 * gg + h, h * D : (h + 1) * D] = 1.0
    return W_lin.astype(np.float16), watt16, ident8, e8rep


def _core_inputs(x_shard, mask_shard, wlin, watt16, ident8, e8rep):
    nb = x_shard.shape[0]
    dtiles = nb * N // DT_ROWS
    xtv = np.ascontiguousarray(
        x_shard.reshape(dtiles, DT_ROWS, F).transpose(0, 2, 1).astype(np.float16)
    )
    mrow = np.where(mask_shard.reshape(1, -1) != 0, 0.0, -1e30).astype(
        ml_dtypes.bfloat16
    )
    nmega = dtiles // MEGA
    # slot-0 feature columns, per pair: xsv[m, f, (j, two, b)] = x[tile, b, 0, f]
    xsv = (
        x_shard[:, 0, :]
        .reshape(nmega, MEGA // 2, 2, 4, F)
        .transpose(0, 4, 1, 2, 3)
        .reshape(nmega, F, 4 * MEGA)
        .astype(np.float16)
    )
    return {
        "xt": xtv,
        "xsrc": np.ascontiguousarray(xsv),
        "wlin": wlin,
        "watt": watt16,
        "ident8": ident8,
        "e8rep": e8rep,
        "maskrow": mrow,
    }


def kernel(x, W_lin, W_att, mask):
    global LAST_RESULT
    x = np.asarray(x, dtype=np.float32)
    W_lin = np.asarray(W_lin, dtype=np.float32)
    W_att = np.asarray(W_att, dtype=np.float32)
    mask = np.asarray(mask)

    wlin, watt16, ident8, e8rep = _host_weights(W_lin, W_att)
    in_maps = []
    for c in range(NCORES):
        in_maps.append(
            _core_inputs(
                x[c * BSHARD : (c + 1) * BSHARD],
                mask[c * BSHARD : (c + 1) * BSHARD],
                wlin,
                watt16,
                ident8,
                e8rep,
            )
        )

    nc = build_nc(DTILES)
    trace = os.environ.get("KERNEL_TRACE", "0") == "1"
    tmpdir = os.environ.get("KERNEL_TRACE_DIR") or None
    res = run_bass_kernel_spmd(
        nc, in_maps, list(range(NCORES)), trace=trace, tmpdir=tmpdir
    )
    LAST_RESULT = res
    return np.concatenate(
        [
            res.results[c]["out"].astype(np.float32).reshape(BSHARD, N, HD)
            for c in range(NCORES)
        ],
        axis=0,
    )

